# revision 29
# baseline (speedup 1.0000x reference)
"""Conditional InstanceNorm2d on 8 Trainium2 NeuronCores (Bass/Tile).

Reference semantics (torch InstanceNorm2d, affine=True, biased var):
    out[b,c,h,w] = (x[b,c,h,w] - mean[b,c]) * rsqrt(var[b,c] + 1e-5)
                   * gamma[style_id[b], c] + beta[style_id[b], c]

Sharding: data-parallel along batch. Each of the 8 cores gets 4 samples,
viewed as [1024 (b,c) rows, 4096 spatial] f32. The kernel is HBM-bound
(must stream all of x in and all of out back); the correctness budget
(rel err < 2e-2 vs f32 reference; fp16 round-trip measures ~6e-4) lets
us move x and out as fp16, halving DMA traffic in both directions.
Stats and the affine params stay f32.

Row r = p*8 + a lives on SBUF partition p, sub-row a, so every DMA tile
[128, k, 4096] fp16 is k*8KiB contiguous per partition line. The compute
is phrased so the engine dependency is ONE-directional (DVE -> ACT);
neither engine ever waits on the other's results mid-stream, which
keeps the whole pipeline DMA-bound. Per tile:
  - DMA load (HWDGE via SP sequencer)
  - DVE: 8x bn_stats(512) + bn_aggr per sub-row -> (mean, var);
    iv = 1/(var+eps) (tensor_scalar_add + reciprocal, [128,k]);
    y = (x - mean) * gamma  (one fused tensor_scalar pass per sub-row,
    in place, fp16)
  - ACT: rstd = sqrt(iv) [128,k]; out = y * rstd + beta (one fused
    activation pass per sub-row, in place, scale=rstd, bias=beta)
  - DMA store (SWDGE via GpSimd)
The [16,256] gamma/beta tables are gathered by style_id on host (32
lookups) as part of input sharding; each core receives its per-row
scale/shift. x is converted f32->fp16 on host; out fp16->f32 on host.
"""

import sys

_REPO = "/opt/trn_rl_repo"
if _REPO not in sys.path:
    sys.path.insert(0, _REPO)

import numpy as np

import concourse.bacc as bacc
import concourse.bass as bass
import concourse.tile as tile
from concourse import mybir
from concourse.bass_utils import run_bass_kernel_spmd
from concourse.bass2jax import (
    _bass_exec_p,
    install_neuronx_cc_hook,
    partition_id_tensor,
)

B, C, H, W = 32, 256, 64, 64
S = 16
N_CORES = 8
B_PER = B // N_CORES  # 4 samples per core
ROWS = B_PER * C  # 1024 (b,c) rows per core
D = H * W  # 4096 spatial elements per row
P = 128  # SBUF partitions
NT = ROWS // P  # 8 sub-rows per partition
CHUNK = 512  # bn_stats hardware max free size
NCHUNK = D // CHUNK  # 8 bn_stats calls per sub-row
EPS = 1e-5
F32 = mybir.dt.float32
F16 = mybir.dt.float16
DT_BYTES = 2  # fp16 x/out streaming dtype

_NC_CACHE = {}


def _build(
    n_reps=1,
    x_bufs=4,
    rows_per_dma=4,
    store_hwdge=True,
    compute=True,
    do_store=True,
    load_split=False,
    store_alt=False,
    layout="pa",
    mode="full",
):
    """Build the per-core kernel. n_reps>1 wraps the body in an in-NEFF
    For_i loop (identical idempotent work) for device-side timing via
    (T(n_reps) - T(1)) / (n_reps - 1).

    rows_per_dma: sub-rows per DMA tile (4 -> 2x 4MiB fp16 tiles).
    store_hwdge: store via scalar-engine HWDGE ring instead of GpSimd SWDGE.
    compute=False / do_store=False / mode=...: probe variants.
    """
    key = (
        n_reps,
        x_bufs,
        rows_per_dma,
        store_hwdge,
        compute,
        do_store,
        load_split,
        store_alt,
        layout,
        mode,
    )
    if key in _NC_CACHE:
        return _NC_CACHE[key]

    k = rows_per_dma
    assert NT % k == 0
    n_tiles = NT // k

    nc = bacc.Bacc(
        "TRN2",
        target_bir_lowering=False,
        debug=False,
        enable_asserts=False,
        num_devices=N_CORES,
    )
    x = nc.dram_tensor("x", [ROWS, D], F16, kind="ExternalInput").ap()
    g = nc.dram_tensor("g", [P, NT], F32, kind="ExternalInput").ap()
    bt = nc.dram_tensor("bt", [P, NT], F32, kind="ExternalInput").ap()
    out = nc.dram_tensor("out", [ROWS, D], F16, kind="ExternalOutput").ap()

    if layout == "pa":
        # row r = p*NT + a: per-partition lines contiguous (k*8KiB)
        xr = x.rearrange("(p a) d -> p a d", p=P)
        outr = out.rearrange("(p a) d -> p a d", p=P)
    else:  # "np": row r = a*P + p -> each k=1 tile is one contiguous block
        xr = x.rearrange("(a p) d -> p a d", p=P)
        outr = out.rearrange("(a p) d -> p a d", p=P)

    with tile.TileContext(nc) as tc:
        with (
            tc.tile_pool(name="xp", bufs=x_bufs) as xp,
            tc.tile_pool(name="sp", bufs=3) as sp,
            tc.tile_pool(name="ones", bufs=1) as ones,
        ):
            g_sb = ones.tile([P, NT], F32, tag="g")
            b_sb = ones.tile([P, NT], F32, tag="b")
            scrap = ones.tile([P, D], F16, tag="scrap")
            nc.gpsimd.dma_start(out=g_sb[:], in_=g)
            nc.gpsimd.dma_start(out=b_sb[:], in_=bt)

            store_eng = nc.scalar if store_hwdge else nc.gpsimd

            def body():
                for j in range(n_tiles):
                    xt = xp.tile([P, k, D], F16, tag="x")
                    if load_split:
                        # halves on the two HWDGE rings (SP + ACT sequencers)
                        h = D // 2
                        nc.sync.dma_start(
                            out=xt[:, :, 0:h],
                            in_=xr[:, j * k : (j + 1) * k, 0:h],
                        )
                        nc.scalar.dma_start(
                            out=xt[:, :, h:D],
                            in_=xr[:, j * k : (j + 1) * k, h:D],
                        )
                    else:
                        nc.sync.dma_start(
                            out=xt[:], in_=xr[:, j * k : (j + 1) * k, :]
                        )
                    if compute and mode != "full":
                        # micro-probes to measure single-engine rates
                        if mode in ("stats",):
                            stats = sp.tile(
                                [P, k, NCHUNK, 6], F32, tag="stats"
                            )
                            mv = sp.tile([P, k, 2], F32, tag="mv")
                            for al in range(k):
                                for c in range(NCHUNK):
                                    nc.vector.bn_stats(
                                        out=stats[:, al, c, :],
                                        in_=xt[:, al, bass.ts(c, CHUNK)],
                                    )
                                nc.vector.bn_aggr(
                                    out=mv[:, al, :], in_=stats[:, al]
                                )
                        elif mode in ("y", "ysep"):
                            if mode == "ysep":
                                yt = xp.tile([P, k, D], F16, tag="y")
                            else:
                                yt = xt
                            for al in range(k):
                                nc.vector.tensor_scalar(
                                    out=yt[:, al, :],
                                    in0=xt[:, al, :],
                                    scalar1=0.5,
                                    scalar2=1.25,
                                    op0=mybir.AluOpType.subtract,
                                    op1=mybir.AluOpType.mult,
                                )
                        elif mode == "sum":
                            acc = sp.tile([P, k], F32, tag="acc")
                            for al in range(k):
                                nc.vector.tensor_scalar(
                                    out=scrap[:],
                                    in0=xt[:, al, :],
                                    scalar1=0.0,
                                    scalar2=None,
                                    op0=mybir.AluOpType.add,
                                    accum_out=acc[:, al : al + 1],
                                )
                        elif mode == "actaccum":
                            acc = sp.tile([P, k], F32, tag="acc")
                            for al in range(k):
                                a = j * k + al
                                nc.scalar.activation(
                                    out=scrap[:],
                                    in_=xt[:, al, :],
                                    func=mybir.ActivationFunctionType.Square,
                                    accum_out=acc[:, al : al + 1],
                                )
                        elif mode in ("act", "actsep"):
                            if mode == "actsep":
                                yt = xp.tile([P, k, D], F16, tag="y")
                            else:
                                yt = xt
                            for al in range(k):
                                a = j * k + al
                                nc.scalar.activation(
                                    out=yt[:, al, :],
                                    in_=xt[:, al, :],
                                    func=mybir.ActivationFunctionType.Identity,
                                    bias=b_sb[:, a : a + 1],
                                    scale=g_sb[:, a : a + 1],
                                )
                    if compute and mode == "full":
                        # DVE: sum via tensor_scalar+accum (4x fp16 mode),
                        # sumsq via scalar_tensor_tensor (x+0)*x +accum (2x)
                        sums = sp.tile([P, k], F32, tag="sums")
                        sq = sp.tile([P, k], F32, tag="sq")
                        for al in range(k):
                            xta = xt[:, al, :]
                            nc.vector.tensor_scalar(
                                out=scrap[:],
                                in0=xta,
                                scalar1=0.0,
                                scalar2=1.0,
                                op0=mybir.AluOpType.add,
                                op1=mybir.AluOpType.mult,
                                accum_out=sums[:, al : al + 1],
                            )
                            nc.vector.scalar_tensor_tensor(
                                out=scrap[:],
                                in0=xta,
                                scalar=0.0,
                                in1=xta,
                                op0=mybir.AluOpType.add,
                                op1=mybir.AluOpType.mult,
                                accum_out=sq[:, al : al + 1],
                            )
                        # mu = sums/D ; ve = sq/D + eps - mu^2 ; iv = 1/ve
                        mu = sp.tile([P, k], F32, tag="mu")
                        nc.vector.tensor_scalar_mul(mu[:], sums[:], 1.0 / D)
                        ve = sp.tile([P, k], F32, tag="ve")
                        nc.vector.tensor_scalar(
                            out=ve[:],
                            in0=sq[:],
                            scalar1=1.0 / D,
                            scalar2=EPS,
                            op0=mybir.AluOpType.mult,
                            op1=mybir.AluOpType.add,
                        )
                        mu2 = sp.tile([P, k], F32, tag="mu2")
                        nc.vector.tensor_mul(mu2[:], mu[:], mu[:])
                        nc.vector.tensor_sub(ve[:], ve[:], mu2[:])
                        nc.vector.reciprocal(out=ve[:], in_=ve[:])
                        # ACT: rstd = sqrt(1/ve)
                        rstd = sp.tile([P, k], F32, tag="rstd")
                        nc.scalar.sqrt(out=rstd[:], in_=ve[:])
                        # DVE: s = g*rstd ; t = b - mu*s
                        s_t = sp.tile([P, k], F32, tag="s")
                        nc.vector.tensor_mul(
                            s_t[:], g_sb[:, j * k : (j + 1) * k], rstd[:]
                        )
                        tt = sp.tile([P, k], F32, tag="t")
                        nc.vector.tensor_mul(tt[:], mu[:], s_t[:])
                        nc.vector.tensor_sub(
                            tt[:], b_sb[:, j * k : (j + 1) * k], tt[:]
                        )
                        # ACT: out = x*s + t, one fused pass per sub-row
                        for al in range(k):
                            xta = xt[:, al, :]
                            nc.scalar.activation(
                                out=xta,
                                in_=xta,
                                func=mybir.ActivationFunctionType.Identity,
                                bias=tt[:, al : al + 1],
                                scale=s_t[:, al : al + 1],
                            )
                    if do_store:
                        se = (
                            (nc.gpsimd if j % 2 == 0 else nc.scalar)
                            if store_alt
                            else store_eng
                        )
                        se.dma_start(
                            out=outr[:, j * k : (j + 1) * k, :], in_=xt[:]
                        )

            if n_reps == 1:
                body()
            else:
                with tc.For_i(0, n_reps, 1):
                    body()

    nc.compile()
    _NC_CACHE[key] = nc
    return nc


def make_in_maps(x, style_id, gamma, beta, layout="pa"):
    """Host-side sharding: batch-split x (cast to fp16), style-gather +
    split gamma/beta."""
    x = np.asarray(x, dtype=np.float32)
    style_id = np.asarray(style_id).astype(np.int64)
    gamma = np.asarray(gamma, dtype=np.float32)
    beta = np.asarray(beta, dtype=np.float32)
    x16 = x.reshape(B, C, H, W).astype(np.float16)
    g_all = gamma[style_id]  # [B, C]
    b_all = beta[style_id]  # [B, C]
    in_maps = []
    for i in range(N_CORES):
        sl = slice(i * B_PER, (i + 1) * B_PER)
        xs = np.ascontiguousarray(x16[sl]).reshape(ROWS, D)
        if layout == "pa":
            # row r = p*NT + a  ->  g_sb[p, a] = g_flat[p*NT + a]
            gs = np.ascontiguousarray(g_all[sl].reshape(P, NT))
            bs = np.ascontiguousarray(b_all[sl].reshape(P, NT))
        else:  # "np": row r = a*P + p
            gs = np.ascontiguousarray(g_all[sl].reshape(NT, P).T)
            bs = np.ascontiguousarray(b_all[sl].reshape(NT, P).T)
        in_maps.append({"x": xs, "g": gs, "bt": bs})
    return in_maps


def run_sharded(in_maps, **kwargs):
    """Run the SPMD kernel; kwargs forwarded to run_bass_kernel_spmd."""
    nc = _build()
    return run_bass_kernel_spmd(nc, in_maps, list(range(N_CORES)), **kwargs)


_EXEC_CACHE = {}


def _prep_executor(nc):
    """Build the jitted 8-core shard_map executor ONCE per nc (mirrors
    run_bass_via_pjrt's multi-core path, but reusable across calls so
    repeated kernel() invocations don't re-trace / recompile)."""
    if id(nc) in _EXEC_CACHE:
        return _EXEC_CACHE[id(nc)]
    import jax
    from jax.experimental.shard_map import shard_map
    from jax.sharding import Mesh, NamedSharding, PartitionSpec

    install_neuronx_cc_hook()

    partition_name = nc.partition_id_tensor.name if nc.partition_id_tensor else None
    in_names, out_names, out_avals, zero_shapes = [], [], [], []
    for alloc in nc.m.functions[0].allocations:
        if not isinstance(alloc, mybir.MemoryLocationSet):
            continue
        name = alloc.memorylocations[0].name
        if alloc.kind == "ExternalInput":
            if name != partition_name:
                in_names.append(name)
        elif alloc.kind == "ExternalOutput":
            out_names.append(name)
            shape = tuple(alloc.tensor_shape)
            dtype = mybir.dt.np(alloc.dtype)
            out_avals.append(jax.core.ShapedArray(shape, dtype))
            zero_shapes.append((shape, dtype))
    all_in_names = in_names + out_names
    if partition_name is not None:
        all_in_names = all_in_names + [partition_name]

    def _body(*args):
        operands = list(args)
        if partition_name is not None:
            operands.append(partition_id_tensor())
        return tuple(
            _bass_exec_p.bind(
                *operands,
                out_avals=tuple(out_avals),
                in_names=tuple(all_in_names),
                out_names=tuple(out_names),
                lowering_input_output_aliases=(),
                sim_require_finite=True,
                sim_require_nnan=True,
                nc=nc,
            )
        )

    devices = jax.devices()[:N_CORES]
    mesh = Mesh(np.asarray(devices), ("core",))
    n_args = len(in_names) + len(out_names)
    fn = jax.jit(
        shard_map(
            _body,
            mesh=mesh,
            in_specs=(PartitionSpec("core"),) * n_args,
            out_specs=(PartitionSpec("core"),) * len(out_names),
            check_rep=False,
        ),
        keep_unused=True,
    )
    sharding = NamedSharding(mesh, PartitionSpec("core"))
    zeros = [
        jax.device_put(np.zeros((N_CORES * s[0], *s[1:]), d), sharding)
        for s, d in zero_shapes
    ]
    entry = (fn, sharding, in_names, zeros)
    _EXEC_CACHE[id(nc)] = entry
    return entry


def kernel(**inputs):
    import jax

    in_maps = make_in_maps(
        inputs["x"], inputs["style_id"], inputs["gamma"], inputs["beta"]
    )
    nc = _build()
    fn, sharding, in_names, zeros = _prep_executor(nc)
    dev_args = [
        jax.device_put(
            np.concatenate([m[name] for m in in_maps], axis=0), sharding
        )
        for name in in_names
    ]
    (out_cat,) = fn(*dev_args, *zeros)
    out_np = np.asarray(out_cat)  # [N_CORES*ROWS, D] fp16
    return out_np.astype(np.float32).reshape(B, C, H, W)


# revision 34
# speedup vs baseline: 1.6573x; 1.6573x over previous
"""Conditional InstanceNorm2d on 8 Trainium2 NeuronCores (Bass/Tile).

Reference semantics (torch InstanceNorm2d, affine=True, biased var):
    out[b,c,h,w] = (x[b,c,h,w] - mean[b,c]) * rsqrt(var[b,c] + 1e-5)
                   * gamma[style_id[b], c] + beta[style_id[b], c]

Sharding: data-parallel along batch. Each of the 8 cores gets 4 samples,
viewed as [1024 (b,c) rows, 4096 spatial] f32. The kernel is HBM-bound
(must stream all of x in and all of out back); the correctness budget
(rel err < 2e-2 vs f32 reference; fp16 round-trip measures ~6e-4) lets
us move x and out as fp16, halving DMA traffic in both directions.
Stats and the affine params stay f32.

Row r = p*8 + a lives on SBUF partition p, sub-row a, so every DMA tile
[128, k, 4096] fp16 is k*8KiB contiguous per partition line. The compute
is phrased so the engine dependency is ONE-directional (DVE -> ACT);
neither engine ever waits on the other's results mid-stream, which
keeps the whole pipeline DMA-bound. Per tile:
  - DMA load (HWDGE via SP sequencer)
  - DVE: 8x bn_stats(512) + bn_aggr per sub-row -> (mean, var);
    iv = 1/(var+eps) (tensor_scalar_add + reciprocal, [128,k]);
    y = (x - mean) * gamma  (one fused tensor_scalar pass per sub-row,
    in place, fp16)
  - ACT: rstd = sqrt(iv) [128,k]; out = y * rstd + beta (one fused
    activation pass per sub-row, in place, scale=rstd, bias=beta)
  - DMA store (SWDGE via GpSimd)
The [16,256] gamma/beta tables are gathered by style_id on host (32
lookups) as part of input sharding; each core receives its per-row
scale/shift. x is converted f32->fp16 on host; out fp16->f32 on host.
"""

import sys

_REPO = "/opt/trn_rl_repo"
if _REPO not in sys.path:
    sys.path.insert(0, _REPO)

import numpy as np

import concourse.bacc as bacc
import concourse.bass as bass
import concourse.tile as tile
from concourse import mybir
from concourse.bass_utils import run_bass_kernel_spmd
from concourse.bass2jax import (
    _bass_exec_p,
    install_neuronx_cc_hook,
    partition_id_tensor,
)

B, C, H, W = 32, 256, 64, 64
S = 16
N_CORES = 8
B_PER = B // N_CORES  # 4 samples per core
ROWS = B_PER * C  # 1024 (b,c) rows per core
D = H * W  # 4096 spatial elements per row
P = 128  # SBUF partitions
NT = ROWS // P  # 8 sub-rows per partition
CHUNK = 512  # bn_stats hardware max free size
NCHUNK = D // CHUNK  # 8 bn_stats calls per sub-row
EPS = 1e-5
F32 = mybir.dt.float32
F16 = mybir.dt.float16
DT_BYTES = 2  # fp16 x/out streaming dtype

_NC_CACHE = {}


def _build(
    n_reps=1,
    x_bufs=4,
    rows_per_dma=4,
    store_hwdge=True,
    compute=True,
    do_store=True,
    load_split=False,
    store_alt=False,
    layout="pa",
    mode="full",
    rsqrt_eng="act",
):
    """Build the per-core kernel. n_reps>1 wraps the body in an in-NEFF
    For_i loop (identical idempotent work) for device-side timing via
    (T(n_reps) - T(1)) / (n_reps - 1).

    rows_per_dma: sub-rows per DMA tile (4 -> 2x 4MiB fp16 tiles).
    store_hwdge: store via scalar-engine HWDGE ring instead of GpSimd SWDGE.
    compute=False / do_store=False / mode=...: probe variants.
    """
    key = (
        n_reps,
        x_bufs,
        rows_per_dma,
        store_hwdge,
        compute,
        do_store,
        load_split,
        store_alt,
        layout,
        mode,
        rsqrt_eng,
    )
    if key in _NC_CACHE:
        return _NC_CACHE[key]

    k = rows_per_dma
    assert NT % k == 0
    n_tiles = NT // k

    nc = bacc.Bacc(
        "TRN2",
        target_bir_lowering=False,
        debug=False,
        enable_asserts=False,
        num_devices=N_CORES,
    )
    x = nc.dram_tensor("x", [ROWS, D], F16, kind="ExternalInput").ap()
    g = nc.dram_tensor("g", [P, NT], F32, kind="ExternalInput").ap()
    bt = nc.dram_tensor("bt", [P, NT], F32, kind="ExternalInput").ap()
    out = nc.dram_tensor("out", [ROWS, D], F16, kind="ExternalOutput").ap()

    if layout == "pa":
        # row r = p*NT + a: per-partition lines contiguous (k*8KiB)
        xr = x.rearrange("(p a) d -> p a d", p=P)
        outr = out.rearrange("(p a) d -> p a d", p=P)
    else:  # "np": row r = a*P + p -> each k=1 tile is one contiguous block
        xr = x.rearrange("(a p) d -> p a d", p=P)
        outr = out.rearrange("(a p) d -> p a d", p=P)

    with tile.TileContext(nc) as tc:
        with (
            tc.tile_pool(name="xp", bufs=x_bufs) as xp,
            tc.tile_pool(name="sp", bufs=3) as sp,
            tc.tile_pool(name="ones", bufs=1) as ones,
        ):
            g_sb = ones.tile([P, NT], F32, tag="g")
            b_sb = ones.tile([P, NT], F32, tag="b")
            scrap = ones.tile([P, D], F16, tag="scrap")
            nc.gpsimd.dma_start(out=g_sb[:], in_=g)
            nc.gpsimd.dma_start(out=b_sb[:], in_=bt)

            store_eng = nc.scalar if store_hwdge else nc.gpsimd

            def body():
                for j in range(n_tiles):
                    xt = xp.tile([P, k, D], F16, tag="x")
                    if load_split:
                        # halves on the two HWDGE rings (SP + ACT sequencers)
                        h = D // 2
                        nc.sync.dma_start(
                            out=xt[:, :, 0:h],
                            in_=xr[:, j * k : (j + 1) * k, 0:h],
                        )
                        nc.scalar.dma_start(
                            out=xt[:, :, h:D],
                            in_=xr[:, j * k : (j + 1) * k, h:D],
                        )
                    else:
                        nc.sync.dma_start(
                            out=xt[:], in_=xr[:, j * k : (j + 1) * k, :]
                        )
                    if compute and mode != "full":
                        # micro-probes to measure single-engine rates
                        if mode in ("stats",):
                            stats = sp.tile(
                                [P, k, NCHUNK, 6], F32, tag="stats"
                            )
                            mv = sp.tile([P, k, 2], F32, tag="mv")
                            for al in range(k):
                                for c in range(NCHUNK):
                                    nc.vector.bn_stats(
                                        out=stats[:, al, c, :],
                                        in_=xt[:, al, bass.ts(c, CHUNK)],
                                    )
                                nc.vector.bn_aggr(
                                    out=mv[:, al, :], in_=stats[:, al]
                                )
                        elif mode in ("y", "ysep"):
                            if mode == "ysep":
                                yt = xp.tile([P, k, D], F16, tag="y")
                            else:
                                yt = xt
                            for al in range(k):
                                nc.vector.tensor_scalar(
                                    out=yt[:, al, :],
                                    in0=xt[:, al, :],
                                    scalar1=0.5,
                                    scalar2=1.25,
                                    op0=mybir.AluOpType.subtract,
                                    op1=mybir.AluOpType.mult,
                                )
                        elif mode in ("sum", "sum32"):
                            acc = sp.tile([P, k], F32, tag="acc")
                            if mode == "sum32":
                                sout = sp.tile([P, D], F32, tag="scr32")
                            else:
                                sout = scrap
                            for al in range(k):
                                nc.vector.tensor_scalar(
                                    out=sout[:],
                                    in0=xt[:, al, :],
                                    scalar1=0.0,
                                    scalar2=1.0,
                                    op0=mybir.AluOpType.add,
                                    op1=mybir.AluOpType.mult,
                                    accum_out=acc[:, al : al + 1],
                                )
                        elif mode in ("stt", "stt32"):
                            acc = sp.tile([P, k], F32, tag="acc")
                            if mode == "stt32":
                                sout = sp.tile([P, D], F32, tag="scr32")
                            else:
                                sout = scrap
                            for al in range(k):
                                nc.vector.scalar_tensor_tensor(
                                    out=sout[:],
                                    in0=xt[:, al, :],
                                    scalar=0.0,
                                    in1=xt[:, al, :],
                                    op0=mybir.AluOpType.add,
                                    op1=mybir.AluOpType.mult,
                                    accum_out=acc[:, al : al + 1],
                                )
                        elif mode == "noaccum":
                            for al in range(k):
                                nc.vector.scalar_tensor_tensor(
                                    out=scrap[:],
                                    in0=xt[:, al, :],
                                    scalar=0.0,
                                    in1=xt[:, al, :],
                                    op0=mybir.AluOpType.add,
                                    op1=mybir.AluOpType.mult,
                                )
                        elif mode == "actaccum":
                            acc = sp.tile([P, k], F32, tag="acc")
                            for al in range(k):
                                a = j * k + al
                                nc.scalar.activation(
                                    out=scrap[:],
                                    in_=xt[:, al, :],
                                    func=mybir.ActivationFunctionType.Square,
                                    accum_out=acc[:, al : al + 1],
                                )
                        elif mode in ("act", "actsep"):
                            if mode == "actsep":
                                yt = xp.tile([P, k, D], F16, tag="y")
                            else:
                                yt = xt
                            for al in range(k):
                                a = j * k + al
                                nc.scalar.activation(
                                    out=yt[:, al, :],
                                    in_=xt[:, al, :],
                                    func=mybir.ActivationFunctionType.Identity,
                                    bias=b_sb[:, a : a + 1],
                                    scale=g_sb[:, a : a + 1],
                                )
                    if compute and mode == "full":
                        # DVE: bn_stats/bn_aggr per sub-row -> (mean, var)
                        stats = sp.tile([P, k, NCHUNK, 6], F32, tag="stats")
                        mv = sp.tile([P, k, 2], F32, tag="mv")
                        for al in range(k):
                            for c in range(NCHUNK):
                                nc.vector.bn_stats(
                                    out=stats[:, al, c, :],
                                    in_=xt[:, al, bass.ts(c, CHUNK)],
                                )
                            nc.vector.bn_aggr(
                                out=mv[:, al, :], in_=stats[:, al]
                            )
                        mu = mv[:, :, 0]
                        ve = sp.tile([P, k], F32, tag="ve")
                        rstd = sp.tile([P, k], F32, tag="rstd")
                        nc.vector.tensor_scalar_add(ve[:], mv[:, :, 1], EPS)
                        if rsqrt_eng == "act":
                            # iv = 1/ve on DVE, rstd = sqrt(iv) on ACT
                            nc.vector.reciprocal(out=ve[:], in_=ve[:])
                            nc.scalar.sqrt(out=rstd[:], in_=ve[:])
                        else:
                            # fast inverse sqrt entirely on DVE:
                            # w0 = magic - (bits(ve) >> 1), 2 Newton steps
                            # w <- w * (1.5 - 0.5*ve*w^2)
                            vei = ve[:].bitcast(mybir.dt.int32)
                            wi = sp.tile([P, k], mybir.dt.int32, tag="wi")
                            nc.vector.tensor_scalar(
                                out=wi[:],
                                in0=vei,
                                scalar1=1,
                                scalar2=-1,
                                op0=mybir.AluOpType.arith_shift_right,
                                op1=mybir.AluOpType.mult,
                            )
                            nc.vector.tensor_scalar_add(
                                wi[:], wi[:], 0x5F3759DF
                            )
                            w = wi[:].bitcast(F32)
                            hv = sp.tile([P, k], F32, tag="hv")
                            nc.vector.tensor_scalar_mul(hv[:], ve[:], -0.5)
                            w2 = sp.tile([P, k], F32, tag="w2")
                            for _ in range(2):
                                nc.vector.tensor_mul(w2[:], w, w)
                                nc.vector.tensor_mul(w2[:], w2[:], hv[:])
                                nc.vector.tensor_scalar_add(
                                    w2[:], w2[:], 1.5
                                )
                                nc.vector.tensor_mul(w, w, w2[:])
                            rstd_ap = w
                        if rsqrt_eng == "act":
                            rstd_ap = rstd[:]
                        # DVE: s = g*rstd ; t = b - mu*s
                        s_t = sp.tile([P, k], F32, tag="s")
                        nc.vector.tensor_mul(
                            s_t[:], g_sb[:, j * k : (j + 1) * k], rstd_ap
                        )
                        tt = sp.tile([P, k], F32, tag="t")
                        nc.vector.tensor_mul(tt[:], mu, s_t[:])
                        nc.vector.tensor_sub(
                            tt[:], b_sb[:, j * k : (j + 1) * k], tt[:]
                        )
                        # ACT: out = x*s + t, one fused pass per sub-row
                        for al in range(k):
                            xta = xt[:, al, :]
                            nc.scalar.activation(
                                out=xta,
                                in_=xta,
                                func=mybir.ActivationFunctionType.Identity,
                                bias=tt[:, al : al + 1],
                                scale=s_t[:, al : al + 1],
                            )
                    if do_store:
                        se = (
                            (nc.gpsimd if j % 2 == 0 else nc.scalar)
                            if store_alt
                            else store_eng
                        )
                        se.dma_start(
                            out=outr[:, j * k : (j + 1) * k, :], in_=xt[:]
                        )

            if n_reps == 1:
                body()
            else:
                with tc.For_i(0, n_reps, 1):
                    body()

    nc.compile()
    _NC_CACHE[key] = nc
    return nc


def make_in_maps(x, style_id, gamma, beta, layout="pa"):
    """Host-side sharding: batch-split x (cast to fp16), style-gather +
    split gamma/beta."""
    x = np.asarray(x, dtype=np.float32)
    style_id = np.asarray(style_id).astype(np.int64)
    gamma = np.asarray(gamma, dtype=np.float32)
    beta = np.asarray(beta, dtype=np.float32)
    x16 = x.reshape(B, C, H, W).astype(np.float16)
    g_all = gamma[style_id]  # [B, C]
    b_all = beta[style_id]  # [B, C]
    in_maps = []
    for i in range(N_CORES):
        sl = slice(i * B_PER, (i + 1) * B_PER)
        xs = np.ascontiguousarray(x16[sl]).reshape(ROWS, D)
        if layout == "pa":
            # row r = p*NT + a  ->  g_sb[p, a] = g_flat[p*NT + a]
            gs = np.ascontiguousarray(g_all[sl].reshape(P, NT))
            bs = np.ascontiguousarray(b_all[sl].reshape(P, NT))
        else:  # "np": row r = a*P + p
            gs = np.ascontiguousarray(g_all[sl].reshape(NT, P).T)
            bs = np.ascontiguousarray(b_all[sl].reshape(NT, P).T)
        in_maps.append({"x": xs, "g": gs, "bt": bs})
    return in_maps


def run_sharded(in_maps, **kwargs):
    """Run the SPMD kernel; kwargs forwarded to run_bass_kernel_spmd."""
    nc = _build()
    return run_bass_kernel_spmd(nc, in_maps, list(range(N_CORES)), **kwargs)


_EXEC_CACHE = {}


def _prep_executor(nc):
    """Build the jitted 8-core shard_map executor ONCE per nc (mirrors
    run_bass_via_pjrt's multi-core path, but reusable across calls so
    repeated kernel() invocations don't re-trace / recompile)."""
    if id(nc) in _EXEC_CACHE:
        return _EXEC_CACHE[id(nc)]
    import jax
    from jax.experimental.shard_map import shard_map
    from jax.sharding import Mesh, NamedSharding, PartitionSpec

    install_neuronx_cc_hook()

    partition_name = nc.partition_id_tensor.name if nc.partition_id_tensor else None
    in_names, out_names, out_avals, zero_shapes = [], [], [], []
    for alloc in nc.m.functions[0].allocations:
        if not isinstance(alloc, mybir.MemoryLocationSet):
            continue
        name = alloc.memorylocations[0].name
        if alloc.kind == "ExternalInput":
            if name != partition_name:
                in_names.append(name)
        elif alloc.kind == "ExternalOutput":
            out_names.append(name)
            shape = tuple(alloc.tensor_shape)
            dtype = mybir.dt.np(alloc.dtype)
            out_avals.append(jax.core.ShapedArray(shape, dtype))
            zero_shapes.append((shape, dtype))
    all_in_names = in_names + out_names
    if partition_name is not None:
        all_in_names = all_in_names + [partition_name]

    def _body(*args):
        operands = list(args)
        if partition_name is not None:
            operands.append(partition_id_tensor())
        return tuple(
            _bass_exec_p.bind(
                *operands,
                out_avals=tuple(out_avals),
                in_names=tuple(all_in_names),
                out_names=tuple(out_names),
                lowering_input_output_aliases=(),
                sim_require_finite=True,
                sim_require_nnan=True,
                nc=nc,
            )
        )

    devices = jax.devices()[:N_CORES]
    mesh = Mesh(np.asarray(devices), ("core",))
    n_args = len(in_names) + len(out_names)
    fn = jax.jit(
        shard_map(
            _body,
            mesh=mesh,
            in_specs=(PartitionSpec("core"),) * n_args,
            out_specs=(PartitionSpec("core"),) * len(out_names),
            check_rep=False,
        ),
        keep_unused=True,
    )
    sharding = NamedSharding(mesh, PartitionSpec("core"))
    zeros = [
        jax.device_put(np.zeros((N_CORES * s[0], *s[1:]), d), sharding)
        for s, d in zero_shapes
    ]
    entry = (fn, sharding, in_names, zeros)
    _EXEC_CACHE[id(nc)] = entry
    return entry


def kernel(**inputs):
    import jax

    in_maps = make_in_maps(
        inputs["x"], inputs["style_id"], inputs["gamma"], inputs["beta"]
    )
    nc = _build()
    fn, sharding, in_names, zeros = _prep_executor(nc)
    dev_args = [
        jax.device_put(
            np.concatenate([m[name] for m in in_maps], axis=0), sharding
        )
        for name in in_names
    ]
    (out_cat,) = fn(*dev_args, *zeros)
    out_np = np.asarray(out_cat)  # [N_CORES*ROWS, D] fp16
    return out_np.astype(np.float32).reshape(B, C, H, W)
